# revision 1
# baseline (speedup 1.0000x reference)
"""Trainium2 Bass kernel for nn_CenterContrastiveLoss.

Problem: loss = label-smoothed CE over [pos, top-50 negs] of f @ centers.T
  f: [2048, 256] f32, centers: [65536, 256] f32, label: [2048] int.

Strategy (8 NeuronCores, tensor-parallel over C=65536):
  - Each core computes S = f @ shard.T for its 8192-column shard in bf16
    (f32 PSUM accumulate), streamed through PSUM in [128 x 1024] tiles.
  - Loop order: for q (4 column chunks of 2048) -> for rt (16 row tiles),
    so the first matmuls need only ~0.8MB of inputs (load hidden).
  - Eviction of PSUM is split to balance engines: per round, 12 of 16
    row-tiles go through ScalarE as exp(S-60) -> bf16 (monotone,
    log-domain precision ~0.004), 4 through VectorE as a fused
    PSUM->f16 grouped max-reduce (raw domain). The exp tiles are folded
    by VectorE pairwise-max at the 2x bf16 rate into 512 bucket-maxes
    per row per core, accumulated across rounds.
  - Host merges 8 x (512 exp + 32 raw) bucket-maxes per row: top-50
    values (S1), the LSE (tail below the buckets is ~1e-9 relative),
    and the positive via value-matching + exact f32 recompute. The
    label-smoothed loss reduces to
      mean(0.9102*lse - 0.9002*pos - 0.0002*S1).
"""

import numpy as np
import ml_dtypes

B, C, D = 2048, 65536, 256
NCORES = 8
CSH = C // NCORES
RT = B // 128              # 16
NQ = 4                     # column chunks (2048 each) per core
QW = CSH // NQ             # 2048
SW = 1024                  # supertile width = 2 PSUM banks
NEXP = 512
NRAW = 32
SHIFT = 60.0

_prog = None


def _build_program():
    import concourse.mybir as mybir
    from concourse import bacc
    from concourse.tile import TileContext
    from contextlib import ExitStack

    bf16 = mybir.dt.bfloat16
    f16 = mybir.dt.float16
    f32 = mybir.dt.float32

    nc = bacc.Bacc("TRN2")
    fT_d = nc.declare_dram_parameter("fT", [2, 128, B], bf16, isOutput=False)
    cT_d = nc.declare_dram_parameter("cT", [2, 128, CSH], bf16, isOutput=False)
    exp_d = nc.declare_dram_parameter("out_exp", [RT, 128, NEXP], bf16, isOutput=True)
    raw_d = nc.declare_dram_parameter("out_raw", [RT, 128, NRAW], f16, isOutput=True)

    def last_exp_q(rt):
        return 2 if rt % NQ == 3 else 3

    def first_exp_q(rt):
        return 1 if rt % NQ == 0 else 0

    with TileContext(nc) as tc, ExitStack() as ctx:
        const = ctx.enter_context(tc.tile_pool(name="const", bufs=1))
        psum = ctx.enter_context(tc.tile_pool(name="psum", bufs=4, space="PSUM"))
        scr = ctx.enter_context(tc.tile_pool(name="scr", bufs=3))
        outp = ctx.enter_context(tc.tile_pool(name="outp", bufs=3))

        fT_t = [const.tile([128, B], bf16, tag=f"fT{k}", name=f"fT{k}")
                for k in range(2)]
        cT_t = [[const.tile([128, QW], bf16, tag=f"cT{k}_{q}",
                            name=f"cT{k}_{q}") for q in range(NQ)]
                for k in range(2)]
        tr_all = const.tile([128, RT * NEXP], bf16, tag="tr_all", name="tr_all")
        bias_t = const.tile([128, 1], f32, tag="bias", name="bias")
        nc.vector.memset(bias_t[:], -SHIFT)
        # critical prefix first: rt0-3 weights + first half of chunk 0
        for k in range(2):
            nc.sync.dma_start(out=fT_t[k][:, 0:512], in_=fT_d[k, :, 0:512])
            nc.sync.dma_start(out=cT_t[k][0][:, 0:SW], in_=cT_d[k, :, 0:SW])
        for k in range(2):
            nc.sync.dma_start(out=cT_t[k][0][:, SW:QW], in_=cT_d[k, :, SW:QW])
            nc.sync.dma_start(out=fT_t[k][:, 512:B], in_=fT_d[k, :, 512:B])
        for q in range(1, NQ):
            for k in range(2):
                nc.sync.dma_start(out=cT_t[k][q][:],
                                  in_=cT_d[k, :, q * QW:(q + 1) * QW])

        for q in range(NQ):
            for rt in range(RT):
                is_raw = (rt % NQ) == q
                tr = tr_all[:, rt * NEXP:(rt + 1) * NEXP]
                if is_raw:
                    raw_t = outp.tile([128, NRAW], f16, tag="raw", name="raw_t")
                else:
                    et = scr.tile([128, QW], bf16, tag="et", name="et")
                for h in range(2):
                    pt = psum.tile([128, SW], f32, tag="pt", name="pt")
                    for k in range(2):
                        lhsT = fT_t[k][:, rt * 128:(rt + 1) * 128]
                        for c in range(2):
                            nc.tensor.matmul(
                                pt[:, c * 512:(c + 1) * 512],
                                lhsT,
                                cT_t[k][q][:, h * SW + c * 512:
                                           h * SW + (c + 1) * 512],
                                start=(k == 0),
                                stop=(k == 1),
                            )
                    if is_raw:
                        nc.vector.tensor_reduce(
                            out=raw_t[:, h * (NRAW // 2):(h + 1) * (NRAW // 2)],
                            in_=pt[:].rearrange("p (g e) -> p g e",
                                                e=SW // (NRAW // 2)),
                            axis=mybir.AxisListType.X,
                            op=mybir.AluOpType.max,
                        )
                    else:
                        nc.scalar.activation(
                            out=et[:, h * SW:(h + 1) * SW],
                            in_=pt[:],
                            func=mybir.ActivationFunctionType.Exp,
                            bias=bias_t[:],
                            scale=1.0,
                        )
                if is_raw:
                    nc.sync.dma_start(out=raw_d[rt], in_=raw_t[:])
                else:
                    fo = scr.tile([128, SW], bf16, tag="fo", name="fo")
                    nc.vector.tensor_max(fo[:], et[:, 0:SW], et[:, SW:2 * SW])
                    if q == first_exp_q(rt):
                        nc.vector.tensor_max(tr, fo[:, 0:NEXP],
                                             fo[:, NEXP:2 * NEXP])
                    else:
                        nc.vector.tensor_max(fo[:, 0:NEXP], fo[:, 0:NEXP],
                                             fo[:, NEXP:2 * NEXP])
                        nc.vector.tensor_max(tr, tr, fo[:, 0:NEXP])
                    if q == last_exp_q(rt):
                        nc.sync.dma_start(out=exp_d[rt], in_=tr)

    nc.finalize()
    return nc


def _get_program():
    global _prog
    if _prog is None:
        _prog = _build_program()
    return _prog


def run_device(in_maps, trace=False, **kw):
    from concourse.bass_utils import run_bass_kernel_spmd

    nc = _get_program()
    return run_bass_kernel_spmd(nc, in_maps, core_ids=list(range(NCORES)),
                                trace=trace, **kw)


def make_in_maps(f, centers, label):
    bf16 = ml_dtypes.bfloat16
    fb = f.astype(bf16)
    cb = centers.astype(bf16)
    fT = np.ascontiguousarray(fb.T).reshape(2, 128, B)
    in_maps = []
    for core in range(NCORES):
        cT = np.ascontiguousarray(
            cb[core * CSH:(core + 1) * CSH].T).reshape(2, 128, CSH)
        in_maps.append({"fT": fT, "cT": cT})
    return in_maps


def postprocess(results, f, centers, label):
    rows = np.arange(B)
    exp_c = np.concatenate(
        [np.asarray(r["out_exp"], dtype=np.float64).reshape(B, NEXP)
         for r in results], axis=1)
    raw_c = np.concatenate(
        [np.asarray(r["out_raw"], dtype=np.float64).reshape(B, NRAW)
         for r in results], axis=1)

    bf16 = ml_dtypes.bfloat16
    fb = f.astype(bf16).astype(np.float32)
    pcb = centers[label].astype(bf16).astype(np.float32)
    pd = np.sum(fb * pcb, axis=1, dtype=np.float32).astype(np.float64)
    pos_f32 = np.einsum("ij,ij->i", f.astype(np.float64),
                        centers[label].astype(np.float64))

    cand_raw = np.concatenate(
        [SHIFT + np.log(np.maximum(exp_c, 1e-300)), raw_c], axis=1)
    win = np.concatenate([np.full(exp_c.shape[1], 0.02),
                          np.full(raw_c.shape[1], 0.12)])
    diff = np.abs(cand_raw - pd[:, None])
    diffm = np.where(diff < win[None, :], diff, np.inf)
    j = np.argmin(diffm, axis=1)
    hit = np.isfinite(diffm[rows, j])
    cand_raw[rows[hit], j[hit]] = -np.inf

    top50 = -np.partition(-cand_raw, 49, axis=1)[:, :50]
    S1 = top50.sum(axis=1)
    se_neg = np.exp(cand_raw - SHIFT,
                    where=np.isfinite(cand_raw),
                    out=np.zeros_like(cand_raw)).sum(axis=1)
    lse = SHIFT + np.log(se_neg + np.exp(pos_f32 - SHIFT))
    loss = (0.9102 * lse - 0.9002 * pos_f32 - 0.0002 * S1).mean()
    return np.array(loss, dtype=np.float32)


def kernel(f, centers, label):
    f = np.asarray(f, dtype=np.float32)
    centers = np.asarray(centers, dtype=np.float32)
    label = np.asarray(label).astype(np.int64)
    in_maps = make_in_maps(f, centers, label)
    try:
        res = run_device(in_maps)
    except Exception:
        # transient runtime flakes (e.g. NRT_EXEC_UNIT_UNRECOVERABLE) have
        # been observed to succeed on immediate retry
        res = run_device(in_maps)
    return postprocess(res.results, f, centers, label)



# revision 2
# speedup vs baseline: 1.1446x; 1.1446x over previous
"""Trainium2 Bass kernel for nn_CenterContrastiveLoss.

Problem: loss = label-smoothed CE over [pos, top-50 negs] of f @ centers.T
  f: [2048, 256] f32, centers: [65536, 256] f32, label: [2048] int.

Strategy (8 NeuronCores, tensor-parallel over C=65536):
  - Inputs quantized to fp8 e4m3; each core computes S = f @ shard.T for its
    8192-column shard with DoubleRow matmuls (K=256 contraction in one pass,
    2x bf16 MAC rate, ~111ns per [256x128x256] MM sustained).
  - PSUM eviction = pairwise column max (bucket=2), split across engines:
    2/3 of [128,2048] PSUM tiles go ScalarE copy->SBUF bf16 then VectorE
    tensor_max at 2x; 1/3 go direct VectorE grouped tensor_reduce (1x).
    Output: per-row noisy bucket maxes in f16, 16MB/core, DMA'd out per
    column-group.
  - Host merges 8 x [2048, 4096] candidate arrays, takes top-96 buckets per
    row, recomputes those ~192 columns exactly in f32, excludes the positive
    column, and evaluates
      loss = mean(0.9102*lse([pos, top50]) - 0.9002*pos - 0.0002*sum(top50)).
    fp8 ranking noise (~0.8) is irrelevant at this margin: sim rel err ~5e-8.
"""

import numpy as np
import ml_dtypes

B, C, D = 2048, 65536, 256
NCORES = 8
CSH = C // NCORES          # 8192
RT = B // 128              # 16
NG = 4                     # column groups of 2048 per core
GW = 2048                  # group width (cols)
OW = GW // 2               # bucket outputs per group (bucket=2)
M_SEL = 96                 # buckets recomputed exactly per row

_prog = None


def _etype(g, rt):
    """Eviction type for tile (g, rt): 'P' (ACT copy + DVE max) or 'Q'
    (DVE grouped reduce). Pattern P,P,Q balances ScalarE vs VectorE."""
    n = g * RT + rt
    return 'Q' if n % 3 == 2 else 'P'


def _build_program():
    import concourse.mybir as mybir
    from concourse import bacc
    from concourse.tile import TileContext
    from contextlib import ExitStack

    bf16 = mybir.dt.bfloat16
    f16 = mybir.dt.float16
    f32 = mybir.dt.float32
    fp8 = mybir.dt.float8e4
    DR = mybir.MatmulPerfMode.DoubleRow

    nc = bacc.Bacc("TRN2")
    fT_d = nc.declare_dram_parameter("fT", [1, 128, 2, B], fp8, isOutput=False)
    cT_d = nc.declare_dram_parameter("cT", [1, 128, 2, CSH], fp8,
                                     isOutput=False)
    out_d = nc.declare_dram_parameter("out", [NG, 128, RT * OW], f16,
                                      isOutput=True)

    with TileContext(nc) as tc, ExitStack() as ctx:
        const = ctx.enter_context(tc.tile_pool(name="const", bufs=1))
        ctp = ctx.enter_context(tc.tile_pool(name="ctp", bufs=2))
        psum = ctx.enter_context(tc.tile_pool(name="psum", bufs=2,
                                              space="PSUM"))
        scp = ctx.enter_context(tc.tile_pool(name="scp", bufs=3))
        stp = ctx.enter_context(tc.tile_pool(name="stp", bufs=2))

        fT_t = const.tile([128, 2, B], fp8, tag="fT", name="fT")

        ct_tiles = []
        for g in range(NG):
            t = ctp.tile([128, 2, GW], fp8, tag="ct", name=f"ct{g}")
            ct_tiles.append(t)
        # prefetch: group 0 first, then f, then the rest
        nc.sync.dma_start(out=ct_tiles[0][:], in_=cT_d[0, :, :, 0:GW])
        nc.sync.dma_start(out=fT_t[:], in_=fT_d[0])
        nc.sync.dma_start(out=ct_tiles[1][:], in_=cT_d[0, :, :, GW:2 * GW])

        for g in range(NG):
            ct = ct_tiles[g]
            if g + 2 < NG:
                nxt = ctp.tile([128, 2, GW], fp8, tag="ct", name=f"ct{g + 2}")
                ct_tiles.append(nxt)
            stage = stp.tile([128, RT * OW], f16, tag="stage", name=f"st{g}")
            for rt in range(RT):
                lhsT = fT_t[:, :, rt * 128:(rt + 1) * 128]
                pt = psum.tile([128, GW], f32, tag="pt", name="pt")
                for j in range(8):
                    nc.tensor.matmul(pt[:, j * 256:(j + 1) * 256], lhsT,
                                     ct[:, :, j * 256:(j + 1) * 256],
                                     start=True, stop=True, perf_mode=DR)
                ob = stage[:, rt * OW:(rt + 1) * OW]
                if _etype(g, rt) == 'P':
                    sc = scp.tile([128, GW], bf16, tag="sc", name="sc")
                    nc.scalar.activation(
                        out=sc[:], in_=pt[:],
                        func=mybir.ActivationFunctionType.Copy, scale=1.0)
                    nc.vector.tensor_max(ob, sc[:, 0:OW], sc[:, OW:GW])
                else:
                    nc.vector.tensor_reduce(
                        out=ob,
                        in_=pt[:].rearrange("p (g e) -> p g e", e=2),
                        axis=mybir.AxisListType.X,
                        op=mybir.AluOpType.max,
                    )
                # prefetch next-next group once the 2-ahead buffer frees
                if rt == 1 and g + 2 < NG:
                    nc.sync.dma_start(
                        out=ct_tiles[g + 2][:],
                        in_=cT_d[0, :, :, (g + 2) * GW:(g + 3) * GW])
            nc.sync.dma_start(out=out_d[g], in_=stage[:])

    nc.finalize()
    return nc


def _get_program():
    global _prog
    if _prog is None:
        _prog = _build_program()
    return _prog


def run_device(in_maps, trace=False, **kw):
    from concourse.bass_utils import run_bass_kernel_spmd

    nc = _get_program()
    return run_bass_kernel_spmd(nc, in_maps, core_ids=list(range(NCORES)),
                                trace=trace, **kw)


def make_in_maps(f, centers, label):
    f8 = ml_dtypes.float8_e4m3
    # fT[p, i, b] = f[b, p + 128*i]
    fq = f.astype(f8)                       # [B, 256]
    fT = np.ascontiguousarray(
        fq.T.reshape(2, 128, B).transpose(1, 0, 2)).reshape(1, 128, 2, B)
    in_maps = []
    for core in range(NCORES):
        sh = centers[core * CSH:(core + 1) * CSH].astype(f8)  # [CSH, 256]
        cT = np.ascontiguousarray(
            sh.T.reshape(2, 128, CSH).transpose(1, 0, 2)).reshape(
                1, 128, 2, CSH)
        in_maps.append({"fT": fT, "cT": cT})
    return in_maps


def _colmap():
    """colmap[rt, k, 2]: global column ids of bucket k (of 32768) for row
    tile rt. Bucket order matches cand assembly: core-major, then g, then j."""
    cm = np.empty((RT, NCORES * NG * OW, 2), np.int64)
    j = np.arange(OW)
    for core in range(NCORES):
        for g in range(NG):
            base = core * CSH + g * GW
            ob = core * NG * OW + g * OW
            for rt in range(RT):
                if _etype(g, rt) == 'P':
                    cm[rt, ob:ob + OW, 0] = base + j
                    cm[rt, ob:ob + OW, 1] = base + OW + j
                else:
                    cm[rt, ob:ob + OW, 0] = base + 2 * j
                    cm[rt, ob:ob + OW, 1] = base + 2 * j + 1
    return cm


def postprocess(results, f, centers, label):
    f = f.astype(np.float32)
    centers_f = centers.astype(np.float32)
    # cand[rt*128+p, core*4096 + g*1024 + j] = out[g, p, rt*1024 + j]
    cands = []
    for r in results:
        o = np.asarray(r["out"]).reshape(NG, 128, RT, OW)
        cands.append(o.transpose(2, 1, 0, 3).reshape(B, NG * OW))
    cand = np.concatenate(cands, axis=1).astype(np.float32)   # [B, 32768]
    cm = _colmap()                                            # [RT, 32768, 2]

    rows = np.arange(B)
    idx = np.argpartition(-cand, M_SEL - 1, axis=1)[:, :M_SEL]  # [B, M]
    cols = cm[rows[:, None] // 128, idx].reshape(B, 2 * M_SEL)  # [B, 2M]
    g = centers_f[cols]                                         # [B, 2M, D]
    Se = np.einsum('bd,bkd->bk', f, g, optimize=True).astype(np.float64)
    Se[cols == label[:, None]] = -np.inf
    top50 = -np.sort(-Se, axis=1)[:, :50]
    pos = np.einsum('bd,bd->b', f.astype(np.float64),
                    centers[label].astype(np.float64))
    preds = np.concatenate([pos[:, None], top50], axis=1)
    m = preds.max(axis=1, keepdims=True)
    lse = (m + np.log(np.exp(preds - m).sum(axis=1, keepdims=True)))[:, 0]
    S1 = top50.sum(axis=1)
    loss = (0.9102 * lse - 0.9002 * pos - 0.0002 * S1).mean()
    return np.array(loss, dtype=np.float32)


def kernel(f, centers, label):
    f = np.asarray(f, dtype=np.float32)
    centers = np.asarray(centers, dtype=np.float32)
    label = np.asarray(label).astype(np.int64)
    in_maps = make_in_maps(f, centers, label)
    try:
        res = run_device(in_maps)
    except Exception:
        # transient runtime flakes (e.g. NRT_EXEC_UNIT_UNRECOVERABLE) have
        # been observed to succeed on immediate retry
        res = run_device(in_maps)
    return postprocess(res.results, f, centers, label)


# revision 3
# speedup vs baseline: 1.5235x; 1.3310x over previous
"""Trainium2 Bass kernel for nn_CenterContrastiveLoss.

Problem: loss = label-smoothed CE over [pos, top-50 negs] of f @ centers.T
  f: [2048, 256] f32, centers: [65536, 256] f32, label: [2048] int.

Strategy (8 NeuronCores, tensor-parallel over C=65536):
  - Inputs quantized to fp8 e4m3; each core computes S = f @ shard.T for its
    8192-column shard with DoubleRow matmuls (K=256 contraction in one MM,
    2x bf16 MAC rate, ~111ns per [256x128x256] MM sustained).
  - Candidate extraction avoids any reduction tree: per [128,1024] PSUM tile,
    even tiles are evicted by ScalarE as a raw fp8 copy (bucket=1), odd tiles
    by VectorE as a pairwise-max grouped reduce (bucket=2, fp8 out). Both are
    single-pass PSUM reads, so the two engines split the 16.8M-element
    eviction evenly (~1.1ns/elem each). PSUM is 4 x [128,1024] tiles deep so
    engine handoffs stay off the critical path.
  - Host merges the 8 x [2048, 6144] candidate arrays, takes the top-128
    noisy candidates per row, recomputes those columns exactly in f32,
    excludes the positive, and evaluates
      loss = mean(0.9102*lse([pos, top50]) - 0.9002*pos - 0.0002*sum(top50)).
    fp8 ranking noise (~1-2 abs) only perturbs selection near rank 50, where
    contributions are ~e^-14 of the lse: simulated end-to-end rel err ~5e-10.
"""

import numpy as np
import ml_dtypes

B, C, D = 2048, 65536, 256
NCORES = 8
CSH = C // NCORES          # 8192
RT = B // 128              # 16
NG = 4                     # column groups of 2048 per core
GW = 2048                  # group width (cols)
TW = 1024                  # psum tile width
OV = TW + TW // 2          # candidate values per (group, row-tile): 1536
M_SEL = 128                # noisy candidates recomputed exactly per row

_prog = None


def _build_program():
    import concourse.mybir as mybir
    from concourse import bacc
    from concourse.tile import TileContext
    from contextlib import ExitStack

    f32 = mybir.dt.float32
    fp8 = mybir.dt.float8e4
    DR = mybir.MatmulPerfMode.DoubleRow

    nc = bacc.Bacc("TRN2")
    fT_d = nc.declare_dram_parameter("fT", [1, 128, 2, B], fp8, isOutput=False)
    cT_d = nc.declare_dram_parameter("cT", [1, 128, 2, CSH], fp8,
                                     isOutput=False)
    out_d = nc.declare_dram_parameter("out", [NG, 128, RT * OV], fp8,
                                      isOutput=True)

    with TileContext(nc) as tc, ExitStack() as ctx:
        const = ctx.enter_context(tc.tile_pool(name="const", bufs=1))
        ctp = ctx.enter_context(tc.tile_pool(name="ctp", bufs=2))
        psum = ctx.enter_context(tc.tile_pool(name="psum", bufs=4,
                                              space="PSUM"))
        stp = ctx.enter_context(tc.tile_pool(name="stp", bufs=2))

        fT_t = const.tile([128, 2, B], fp8, tag="fT", name="fT")

        ct_tiles = []
        for g in range(2):
            ct_tiles.append(ctp.tile([128, 2, GW], fp8, tag="ct",
                                     name=f"ct{g}"))
        # prefetch: group 0 first, then f, then group 1
        nc.sync.dma_start(out=ct_tiles[0][:], in_=cT_d[0, :, :, 0:GW])
        nc.sync.dma_start(out=fT_t[:], in_=fT_d[0])
        nc.sync.dma_start(out=ct_tiles[1][:], in_=cT_d[0, :, :, GW:2 * GW])

        for g in range(NG):
            ct = ct_tiles[g]
            if g + 2 < NG:
                ct_tiles.append(ctp.tile([128, 2, GW], fp8, tag="ct",
                                         name=f"ct{g + 2}"))
            stage = stp.tile([128, RT * OV], fp8, tag="stage", name=f"st{g}")
            for rt in range(RT):
                lhsT = fT_t[:, :, rt * 128:(rt + 1) * 128]
                for half in range(2):
                    pt = psum.tile([128, TW], f32, tag="pt", name="pt")
                    cb = half * TW
                    for j in range(4):
                        nc.tensor.matmul(
                            pt[:, j * 256:(j + 1) * 256], lhsT,
                            ct[:, :, cb + j * 256:cb + (j + 1) * 256],
                            start=True, stop=True, perf_mode=DR)
                    if half == 0:
                        nc.scalar.activation(
                            out=stage[:, rt * OV:rt * OV + TW], in_=pt[:],
                            func=mybir.ActivationFunctionType.Copy, scale=1.0)
                    else:
                        nc.vector.tensor_reduce(
                            out=stage[:, rt * OV + TW:(rt + 1) * OV],
                            in_=pt[:].rearrange("p (g e) -> p g e", e=2),
                            axis=mybir.AxisListType.X,
                            op=mybir.AluOpType.max,
                        )
                # prefetch next-next group once its buffer frees
                if rt == 1 and g + 2 < NG:
                    nc.sync.dma_start(
                        out=ct_tiles[g + 2][:],
                        in_=cT_d[0, :, :, (g + 2) * GW:(g + 3) * GW])
            nc.sync.dma_start(out=out_d[g], in_=stage[:])

    nc.finalize()
    return nc


def _get_program():
    global _prog
    if _prog is None:
        _prog = _build_program()
    return _prog


def run_device(in_maps, trace=False, **kw):
    from concourse.bass_utils import run_bass_kernel_spmd

    nc = _get_program()
    return run_bass_kernel_spmd(nc, in_maps, core_ids=list(range(NCORES)),
                                trace=trace, **kw)


def make_in_maps(f, centers, label):
    f8 = ml_dtypes.float8_e4m3
    # fT[p, i, b] = f[b, p + 128*i]
    fq = f.astype(f8)                       # [B, 256]
    fT = np.ascontiguousarray(
        fq.T.reshape(2, 128, B).transpose(1, 0, 2)).reshape(1, 128, 2, B)
    in_maps = []
    for core in range(NCORES):
        sh = centers[core * CSH:(core + 1) * CSH].astype(f8)  # [CSH, 256]
        cT = np.ascontiguousarray(
            sh.T.reshape(2, 128, CSH).transpose(1, 0, 2)).reshape(
                1, 128, 2, CSH)
        in_maps.append({"fT": fT, "cT": cT})
    return in_maps


def _colmap():
    """colmap[k, 2]: global column ids for candidate k (of 8*NG*OV).
    Candidate order: core-major, then g, then [1024 singletons | 512 pairs].
    Second col is -1 for singletons. Same for every row tile."""
    cm = np.empty((NCORES * NG * OV, 2), np.int64)
    for core in range(NCORES):
        for g in range(NG):
            base = core * CSH + g * GW
            ob = (core * NG + g) * OV
            cm[ob:ob + TW, 0] = base + np.arange(TW)
            cm[ob:ob + TW, 1] = -1
            cm[ob + TW:ob + OV, 0] = base + TW + 2 * np.arange(TW // 2)
            cm[ob + TW:ob + OV, 1] = base + TW + 2 * np.arange(TW // 2) + 1
    return cm


def postprocess(results, f, centers, label):
    f32f = f.astype(np.float32)
    # cand[rt*128+p, (core*NG+g)*OV + j] = out[g, p, rt*OV + j]
    cands = []
    for r in results:
        o = np.asarray(r["out"]).astype(np.float16)  # fp8 -> f16 widen
        o = o.reshape(NG, 128, RT, OV)
        cands.append(o.transpose(2, 1, 0, 3).reshape(B, NG * OV))
    cand = np.concatenate(cands, axis=1).astype(np.float32)  # [B, 49152]
    cm = _colmap()

    idx = np.argpartition(-cand, M_SEL - 1, axis=1)[:, :M_SEL]  # [B, M]
    cols = cm[idx].reshape(B, 2 * M_SEL)                        # [B, 2M]
    valid = cols >= 0
    cols_c = np.where(valid, cols, 0)
    g = centers[cols_c]                                         # [B, 2M, D]
    Se = np.einsum('bd,bkd->bk', f32f, g.astype(np.float32),
                   optimize=True).astype(np.float64)
    Se[~valid] = -np.inf
    Se[cols_c == label[:, None]] = -np.inf
    top50 = -np.sort(-Se, axis=1)[:, :50]
    pos = np.einsum('bd,bd->b', f.astype(np.float64),
                    centers[label].astype(np.float64))
    preds = np.concatenate([pos[:, None], top50], axis=1)
    m = preds.max(axis=1, keepdims=True)
    lse = (m + np.log(np.exp(preds - m).sum(axis=1, keepdims=True)))[:, 0]
    S1 = top50.sum(axis=1)
    loss = (0.9102 * lse - 0.9002 * pos - 0.0002 * S1).mean()
    return np.array(loss, dtype=np.float32)


def kernel(f, centers, label):
    f = np.asarray(f, dtype=np.float32)
    centers = np.asarray(centers, dtype=np.float32)
    label = np.asarray(label).astype(np.int64)
    in_maps = make_in_maps(f, centers, label)
    try:
        res = run_device(in_maps)
    except Exception:
        # transient runtime flakes (e.g. NRT_EXEC_UNIT_UNRECOVERABLE) have
        # been observed to succeed on immediate retry
        res = run_device(in_maps)
    return postprocess(res.results, f, centers, label)


# revision 5
# speedup vs baseline: 1.6473x; 1.0813x over previous
"""Trainium2 Bass kernel for nn_CenterContrastiveLoss.

Problem: loss = label-smoothed CE over [pos, top-50 negs] of f @ centers.T
  f: [2048, 256] f32, centers: [65536, 256] f32, label: [2048] int.

Strategy (8 NeuronCores, tensor-parallel over C=65536):
  - Inputs quantized to fp8 e4m3; each core computes S = f @ shard.T for its
    8192-column shard with DoubleRow matmuls (K=256 contraction in one MM,
    2x bf16 MAC rate, ~111ns per [256x128x256] MM sustained).
  - Candidate extraction avoids any reduction tree: per [128,1024] PSUM tile,
    even tiles are evicted by ScalarE as a raw fp8 copy (bucket=1), odd tiles
    by VectorE as a pairwise-max grouped reduce (bucket=2, fp8 out). Both are
    single-pass PSUM reads, so the two engines split the 16.8M-element
    eviction evenly (~1.1ns/elem each). PSUM is 4 x [128,1024] tiles deep so
    engine handoffs stay off the critical path.
  - Host merges the 8 x [2048, 6144] candidate arrays, takes the top-128
    noisy candidates per row, recomputes those columns exactly in f32,
    excludes the positive, and evaluates
      loss = mean(0.9102*lse([pos, top50]) - 0.9002*pos - 0.0002*sum(top50)).
    fp8 ranking noise (~1-2 abs) only perturbs selection near rank 50, where
    contributions are ~e^-14 of the lse: simulated end-to-end rel err ~5e-10.
"""

import numpy as np
import ml_dtypes

B, C, D = 2048, 65536, 256
NCORES = 8
CSH = C // NCORES          # 8192
RT = B // 128              # 16
NG = 4                     # column groups of 2048 per core
GW = 2048                  # group width (cols)
TW = 1024                  # psum tile width
OV = TW + TW // 2          # candidate values per (group, row-tile): 1536
M_SEL = 128                # noisy candidates recomputed exactly per row

_prog = None


def _build_program():
    import concourse.mybir as mybir
    from concourse import bacc
    from concourse.tile import TileContext
    from contextlib import ExitStack

    f32 = mybir.dt.float32
    fp8 = mybir.dt.float8e4
    DR = mybir.MatmulPerfMode.DoubleRow

    nc = bacc.Bacc("TRN2")
    fT_d = nc.declare_dram_parameter("fT", [1, 128, 2, B], fp8, isOutput=False)
    cT_d = nc.declare_dram_parameter("cT", [1, 128, 2, CSH], fp8,
                                     isOutput=False)
    out_d = nc.declare_dram_parameter("out", [NG, 128, RT * OV], fp8,
                                      isOutput=True)

    with TileContext(nc) as tc, ExitStack() as ctx:
        const = ctx.enter_context(tc.tile_pool(name="const", bufs=1))
        ctp = ctx.enter_context(tc.tile_pool(name="ctp", bufs=2))
        psum = ctx.enter_context(tc.tile_pool(name="psum", bufs=4,
                                              space="PSUM"))
        stp = ctx.enter_context(tc.tile_pool(name="stp", bufs=2))

        fT_t = const.tile([128, 2, B], fp8, tag="fT", name="fT")

        ct_tiles = []
        for g in range(2):
            ct_tiles.append(ctp.tile([128, 2, GW], fp8, tag="ct",
                                     name=f"ct{g}"))
        # prefetch in need-order: rt0's f block, ct group0 halves, rest of f
        nc.sync.dma_start(out=fT_t[:, :, 0:128], in_=fT_d[0, :, :, 0:128])
        nc.sync.dma_start(out=ct_tiles[0][:, :, 0:TW],
                          in_=cT_d[0, :, :, 0:TW])
        nc.sync.dma_start(out=ct_tiles[0][:, :, TW:GW],
                          in_=cT_d[0, :, :, TW:GW])
        nc.sync.dma_start(out=fT_t[:, :, 128:B], in_=fT_d[0, :, :, 128:B])
        nc.sync.dma_start(out=ct_tiles[1][:], in_=cT_d[0, :, :, GW:2 * GW])

        for g in range(NG):
            ct = ct_tiles[g]
            if g + 2 < NG:
                ct_tiles.append(ctp.tile([128, 2, GW], fp8, tag="ct",
                                         name=f"ct{g + 2}"))
            stage = stp.tile([128, RT * OV], fp8, tag="stage", name=f"st{g}")
            for rt in range(RT):
                lhsT = fT_t[:, :, rt * 128:(rt + 1) * 128]
                for half in range(2):
                    pt = psum.tile([128, TW], f32, tag="pt", name="pt")
                    cb = half * TW
                    for j in range(4):
                        nc.tensor.matmul(
                            pt[:, j * 256:(j + 1) * 256], lhsT,
                            ct[:, :, cb + j * 256:cb + (j + 1) * 256],
                            start=True, stop=True, perf_mode=DR)
                    if half == 0:
                        nc.scalar.activation(
                            out=stage[:, rt * OV:rt * OV + TW], in_=pt[:],
                            func=mybir.ActivationFunctionType.Copy, scale=1.0)
                    else:
                        nc.vector.tensor_reduce(
                            out=stage[:, rt * OV + TW:(rt + 1) * OV],
                            in_=pt[:].rearrange("p (g e) -> p g e", e=2),
                            axis=mybir.AxisListType.X,
                            op=mybir.AluOpType.max,
                        )
                # prefetch next-next group once its buffer frees
                if rt == 1 and g + 2 < NG:
                    nc.sync.dma_start(
                        out=ct_tiles[g + 2][:],
                        in_=cT_d[0, :, :, (g + 2) * GW:(g + 3) * GW])
                nc.sync.dma_start(out=out_d[g, :, rt * OV:(rt + 1) * OV],
                                  in_=stage[:, rt * OV:(rt + 1) * OV])

    nc.finalize()
    return nc


def _get_program():
    global _prog
    if _prog is None:
        _prog = _build_program()
    return _prog


def run_device(in_maps, trace=False, **kw):
    from concourse.bass_utils import run_bass_kernel_spmd

    nc = _get_program()
    return run_bass_kernel_spmd(nc, in_maps, core_ids=list(range(NCORES)),
                                trace=trace, **kw)


def make_in_maps(f, centers, label):
    f8 = ml_dtypes.float8_e4m3
    # fT[p, i, b] = f[b, p + 128*i]
    fq = f.astype(f8)                       # [B, 256]
    fT = np.ascontiguousarray(
        fq.T.reshape(2, 128, B).transpose(1, 0, 2)).reshape(1, 128, 2, B)
    in_maps = []
    for core in range(NCORES):
        sh = centers[core * CSH:(core + 1) * CSH].astype(f8)  # [CSH, 256]
        cT = np.ascontiguousarray(
            sh.T.reshape(2, 128, CSH).transpose(1, 0, 2)).reshape(
                1, 128, 2, CSH)
        in_maps.append({"fT": fT, "cT": cT})
    return in_maps


def _colmap():
    """colmap[k, 2]: global column ids for candidate k (of 8*NG*OV).
    Candidate order: core-major, then g, then [1024 singletons | 512 pairs].
    Second col is -1 for singletons. Same for every row tile."""
    cm = np.empty((NCORES * NG * OV, 2), np.int64)
    for core in range(NCORES):
        for g in range(NG):
            base = core * CSH + g * GW
            ob = (core * NG + g) * OV
            cm[ob:ob + TW, 0] = base + np.arange(TW)
            cm[ob:ob + TW, 1] = -1
            cm[ob + TW:ob + OV, 0] = base + TW + 2 * np.arange(TW // 2)
            cm[ob + TW:ob + OV, 1] = base + TW + 2 * np.arange(TW // 2) + 1
    return cm


def postprocess(results, f, centers, label):
    f32f = f.astype(np.float32)
    # cand[rt*128+p, (core*NG+g)*OV + j] = out[g, p, rt*OV + j]
    cands = []
    for r in results:
        o = np.asarray(r["out"]).astype(np.float16)  # fp8 -> f16 widen
        o = o.reshape(NG, 128, RT, OV)
        cands.append(o.transpose(2, 1, 0, 3).reshape(B, NG * OV))
    cand = np.concatenate(cands, axis=1).astype(np.float32)  # [B, 49152]
    cm = _colmap()

    idx = np.argpartition(-cand, M_SEL - 1, axis=1)[:, :M_SEL]  # [B, M]
    cols = cm[idx].reshape(B, 2 * M_SEL)                        # [B, 2M]
    valid = cols >= 0
    cols_c = np.where(valid, cols, 0)
    g = centers[cols_c]                                         # [B, 2M, D]
    Se = np.einsum('bd,bkd->bk', f32f, g.astype(np.float32),
                   optimize=True).astype(np.float64)
    Se[~valid] = -np.inf
    Se[cols_c == label[:, None]] = -np.inf
    top50 = -np.sort(-Se, axis=1)[:, :50]
    pos = np.einsum('bd,bd->b', f.astype(np.float64),
                    centers[label].astype(np.float64))
    preds = np.concatenate([pos[:, None], top50], axis=1)
    m = preds.max(axis=1, keepdims=True)
    lse = (m + np.log(np.exp(preds - m).sum(axis=1, keepdims=True)))[:, 0]
    S1 = top50.sum(axis=1)
    loss = (0.9102 * lse - 0.9002 * pos - 0.0002 * S1).mean()
    return np.array(loss, dtype=np.float32)


def kernel(f, centers, label):
    f = np.asarray(f, dtype=np.float32)
    centers = np.asarray(centers, dtype=np.float32)
    label = np.asarray(label).astype(np.int64)
    in_maps = make_in_maps(f, centers, label)
    try:
        res = run_device(in_maps)
    except Exception:
        # transient runtime flakes (e.g. NRT_EXEC_UNIT_UNRECOVERABLE) have
        # been observed to succeed on immediate retry
        res = run_device(in_maps)
    return postprocess(res.results, f, centers, label)


# revision 6
# speedup vs baseline: 1.6560x; 1.0053x over previous
"""Trainium2 Bass kernel for nn_CenterContrastiveLoss.

Problem: loss = label-smoothed CE over [pos, top-50 negs] of f @ centers.T
  f: [2048, 256] f32, centers: [65536, 256] f32, label: [2048] int.

Strategy (8 NeuronCores, tensor-parallel over C=65536):
  - Inputs quantized to fp8 e4m3; each core computes S = f @ shard.T for its
    8192-column shard with DoubleRow matmuls (K=256 contraction in one MM,
    2x bf16 MAC rate, ~111ns per [256x128x256] MM sustained).
  - Candidate extraction avoids any reduction tree: per [128,1024] PSUM tile,
    even tiles are evicted by ScalarE as a raw fp8 copy (bucket=1), odd tiles
    by VectorE as a pairwise-max grouped reduce (bucket=2, fp8 out). Both are
    single-pass PSUM reads, so the two engines split the 16.8M-element
    eviction evenly (~1.1ns/elem each). PSUM is 4 x [128,1024] tiles deep so
    engine handoffs stay off the critical path.
  - Host merges the 8 x [2048, 6144] candidate arrays, takes the top-128
    noisy candidates per row, recomputes those columns exactly in f32,
    excludes the positive, and evaluates
      loss = mean(0.9102*lse([pos, top50]) - 0.9002*pos - 0.0002*sum(top50)).
    fp8 ranking noise (~1-2 abs) only perturbs selection near rank 50, where
    contributions are ~e^-14 of the lse: simulated end-to-end rel err ~5e-10.
"""

import numpy as np
import ml_dtypes

B, C, D = 2048, 65536, 256
NCORES = 8
CSH = C // NCORES          # 8192
RT = B // 128              # 16
NG = 4                     # column groups of 2048 per core
GW = 2048                  # group width (cols)
TW = 1024                  # psum tile width
OV = TW + TW // 2          # candidate values per (group, row-tile): 1536
M_SEL = 128                # noisy candidates recomputed exactly per row

_prog = None


def _build_program():
    import concourse.mybir as mybir
    from concourse import bacc
    from concourse.tile import TileContext
    from contextlib import ExitStack

    f32 = mybir.dt.float32
    fp8 = mybir.dt.float8e4
    DR = mybir.MatmulPerfMode.DoubleRow

    nc = bacc.Bacc("TRN2")
    fT_d = nc.declare_dram_parameter("fT", [1, 128, 2, B], fp8, isOutput=False)
    cT_d = nc.declare_dram_parameter("cT", [1, 128, 2, CSH], fp8,
                                     isOutput=False)
    out_d = nc.declare_dram_parameter("out", [NG, 128, RT * OV], fp8,
                                      isOutput=True)

    with TileContext(nc) as tc, ExitStack() as ctx:
        const = ctx.enter_context(tc.tile_pool(name="const", bufs=1))
        ctp = ctx.enter_context(tc.tile_pool(name="ctp", bufs=2))
        psum = ctx.enter_context(tc.tile_pool(name="psum", bufs=4,
                                              space="PSUM"))
        stp = ctx.enter_context(tc.tile_pool(name="stp", bufs=2))

        fT_t = const.tile([128, 2, B], fp8, tag="fT", name="fT")

        ct_tiles = []
        for g in range(2):
            ct_tiles.append(ctp.tile([128, 2, GW], fp8, tag="ct",
                                     name=f"ct{g}"))
        # prefetch in need-order; split across both HWDGE queues so the
        # first matmul's inputs (rt0 f block + first 512 ct cols) land fast
        nc.scalar.dma_start(out=fT_t[:, :, 0:128], in_=fT_d[0, :, :, 0:128])
        nc.sync.dma_start(out=ct_tiles[0][:, :, 0:512],
                          in_=cT_d[0, :, :, 0:512])
        nc.sync.dma_start(out=ct_tiles[0][:, :, 512:TW],
                          in_=cT_d[0, :, :, 512:TW])
        nc.sync.dma_start(out=ct_tiles[0][:, :, TW:GW],
                          in_=cT_d[0, :, :, TW:GW])
        nc.scalar.dma_start(out=fT_t[:, :, 128:B], in_=fT_d[0, :, :, 128:B])
        nc.sync.dma_start(out=ct_tiles[1][:], in_=cT_d[0, :, :, GW:2 * GW])

        for g in range(NG):
            ct = ct_tiles[g]
            if g + 2 < NG:
                ct_tiles.append(ctp.tile([128, 2, GW], fp8, tag="ct",
                                         name=f"ct{g + 2}"))
            stage = stp.tile([128, RT * OV], fp8, tag="stage", name=f"st{g}")
            for rt in range(RT):
                lhsT = fT_t[:, :, rt * 128:(rt + 1) * 128]
                for half in range(2):
                    pt = psum.tile([128, TW], f32, tag="pt", name="pt")
                    cb = half * TW
                    for j in range(4):
                        nc.tensor.matmul(
                            pt[:, j * 256:(j + 1) * 256], lhsT,
                            ct[:, :, cb + j * 256:cb + (j + 1) * 256],
                            start=True, stop=True, perf_mode=DR)
                    if half == 0:
                        nc.scalar.activation(
                            out=stage[:, rt * OV:rt * OV + TW], in_=pt[:],
                            func=mybir.ActivationFunctionType.Copy, scale=1.0)
                    else:
                        nc.vector.tensor_reduce(
                            out=stage[:, rt * OV + TW:(rt + 1) * OV],
                            in_=pt[:].rearrange("p (g e) -> p g e", e=2),
                            axis=mybir.AxisListType.X,
                            op=mybir.AluOpType.max,
                        )
                # prefetch next-next group once its buffer frees
                if rt == 1 and g + 2 < NG:
                    nc.sync.dma_start(
                        out=ct_tiles[g + 2][:],
                        in_=cT_d[0, :, :, (g + 2) * GW:(g + 3) * GW])
                nc.sync.dma_start(out=out_d[g, :, rt * OV:(rt + 1) * OV],
                                  in_=stage[:, rt * OV:(rt + 1) * OV])

    nc.finalize()
    return nc


def _get_program():
    global _prog
    if _prog is None:
        _prog = _build_program()
    return _prog


def run_device(in_maps, trace=False, **kw):
    from concourse.bass_utils import run_bass_kernel_spmd

    nc = _get_program()
    return run_bass_kernel_spmd(nc, in_maps, core_ids=list(range(NCORES)),
                                trace=trace, **kw)


def make_in_maps(f, centers, label):
    f8 = ml_dtypes.float8_e4m3
    # fT[p, i, b] = f[b, p + 128*i]
    fq = f.astype(f8)                       # [B, 256]
    fT = np.ascontiguousarray(
        fq.T.reshape(2, 128, B).transpose(1, 0, 2)).reshape(1, 128, 2, B)
    in_maps = []
    for core in range(NCORES):
        sh = centers[core * CSH:(core + 1) * CSH].astype(f8)  # [CSH, 256]
        cT = np.ascontiguousarray(
            sh.T.reshape(2, 128, CSH).transpose(1, 0, 2)).reshape(
                1, 128, 2, CSH)
        in_maps.append({"fT": fT, "cT": cT})
    return in_maps


def _colmap():
    """colmap[k, 2]: global column ids for candidate k (of 8*NG*OV).
    Candidate order: core-major, then g, then [1024 singletons | 512 pairs].
    Second col is -1 for singletons. Same for every row tile."""
    cm = np.empty((NCORES * NG * OV, 2), np.int64)
    for core in range(NCORES):
        for g in range(NG):
            base = core * CSH + g * GW
            ob = (core * NG + g) * OV
            cm[ob:ob + TW, 0] = base + np.arange(TW)
            cm[ob:ob + TW, 1] = -1
            cm[ob + TW:ob + OV, 0] = base + TW + 2 * np.arange(TW // 2)
            cm[ob + TW:ob + OV, 1] = base + TW + 2 * np.arange(TW // 2) + 1
    return cm


def postprocess(results, f, centers, label):
    f32f = f.astype(np.float32)
    # cand[rt*128+p, (core*NG+g)*OV + j] = out[g, p, rt*OV + j]
    cands = []
    for r in results:
        o = np.asarray(r["out"]).astype(np.float16)  # fp8 -> f16 widen
        o = o.reshape(NG, 128, RT, OV)
        cands.append(o.transpose(2, 1, 0, 3).reshape(B, NG * OV))
    cand = np.concatenate(cands, axis=1).astype(np.float32)  # [B, 49152]
    cm = _colmap()

    idx = np.argpartition(-cand, M_SEL - 1, axis=1)[:, :M_SEL]  # [B, M]
    cols = cm[idx].reshape(B, 2 * M_SEL)                        # [B, 2M]
    valid = cols >= 0
    cols_c = np.where(valid, cols, 0)
    g = centers[cols_c]                                         # [B, 2M, D]
    Se = np.einsum('bd,bkd->bk', f32f, g.astype(np.float32),
                   optimize=True).astype(np.float64)
    Se[~valid] = -np.inf
    Se[cols_c == label[:, None]] = -np.inf
    top50 = -np.sort(-Se, axis=1)[:, :50]
    pos = np.einsum('bd,bd->b', f.astype(np.float64),
                    centers[label].astype(np.float64))
    preds = np.concatenate([pos[:, None], top50], axis=1)
    m = preds.max(axis=1, keepdims=True)
    lse = (m + np.log(np.exp(preds - m).sum(axis=1, keepdims=True)))[:, 0]
    S1 = top50.sum(axis=1)
    loss = (0.9102 * lse - 0.9002 * pos - 0.0002 * S1).mean()
    return np.array(loss, dtype=np.float32)


def kernel(f, centers, label):
    f = np.asarray(f, dtype=np.float32)
    centers = np.asarray(centers, dtype=np.float32)
    label = np.asarray(label).astype(np.int64)
    in_maps = make_in_maps(f, centers, label)
    try:
        res = run_device(in_maps)
    except Exception:
        # transient runtime flakes (e.g. NRT_EXEC_UNIT_UNRECOVERABLE) have
        # been observed to succeed on immediate retry
        res = run_device(in_maps)
    return postprocess(res.results, f, centers, label)
